# revision 9
# baseline (speedup 1.0000x reference)
"""Grouped-experts MoE FFN (SwiGLU) kernel for Trainium2, 8 NeuronCores.

Strategy: expert-parallel with host-side packing.  Token counts per expert
are data, so the host chops each expert's contiguous token block into
chunks and bins them into a uniform per-core "slot" structure
(S slots per core, compile-time sizes).  Every slot is bound to exactly
one expert; the expert's (host-pre-permuted) weights are plain kernel
inputs, so the SPMD program is identical on all 8 cores and needs no
device-side transposes or gather/scatter.

All matmul operands are bf16 (PSUM accumulates fp32): same PE throughput
as fp32r (1 col/cycle) but half the DMA traffic and SBUF footprint, and
the compiler's fast-weight-load path (disabled for fp32) hides LDWEIGHTS
under the matmul stream.

Per-core device program, per slot of capacity L (tokens):
  phase A: for each 128-row h-chunk (22 of them):
      psum1[128,L] = sum_dc w1r[hc,dc].T @ xT[dc]     (bf16 matmuls)
      psum3[128,L] = sum_dc w3r[hc,dc].T @ xT[dc]
      h[hc] = silu(psum1) * psum3                     (ACT + DVE, bf16 out)
  phase B: for each 128-row d-chunk (8):
      po[128,L] = sum_hc w2r[dc,hc].T @ h[hc]
      DMA po -> outT[dc]                              ([D,L] layout, fp32)

Host then transposes each slot's [D, L] output back and scatters into the
full [T, D] result (padding rows stay zero).
"""

import itertools
import numpy as np
from functools import lru_cache

import ml_dtypes

E, D, H, T = 8, 1024, 2816, 16384
P = 128
DC, HC = D // P, H // P  # 8, 22
NCORES = 8
BF16 = ml_dtypes.bfloat16

# Slot sizes: multiples of 16.  Exclude (512, 704): their trailing matmul
# chunk (L - 512 in [16, 176]) would be shorter than an LDWEIGHTS.
_SIZES = [s for s in range(192, 1025, 16) if not (512 < s < 704)]
_GRAN = 16
_FALLBACK = (1024, 1024, 1024)  # feasible for any counts with sum <= T


def _try_assign(Ls, counts, node_cap=500000):
    """Greedy: assign experts (largest first) chunk counts per size class
    (<= NCORES slots per class), minimizing each expert's over-capacity via
    bounded branch&bound.  Returns per-expert chunk-count tuples or None."""
    S = len(Ls)
    avail = [NCORES] * S
    out = [None] * len(counts)
    order = sorted(range(len(counts)), key=lambda e: -counts[e])
    nodes = [0]
    for e in order:
        g = counts[e]
        best = None

        def rec(k, ns, cap):
            nonlocal best
            nodes[0] += 1
            if nodes[0] > node_cap:
                return
            if cap >= g:
                if best is None or cap - g < best[0]:
                    best = (cap - g, tuple(ns))
                return
            if k == S:
                return
            if best is not None and best[0] == 0:
                return
            if cap + sum(avail[j] * Ls[j] for j in range(k, S)) < g:
                return
            for n in range(avail[k], -1, -1):
                rec(k + 1, ns + [n], cap + n * Ls[k])

        rec(0, [], 0)
        if best is None:
            return None
        ns = best[1] + (0,) * (S - len(best[1]))
        for k in range(S):
            avail[k] -= ns[k]
        out[e] = ns
    return out


def _multisets_summing(sizes_desc, S, target):
    """All descending multisets of length S from sizes_desc with given sum."""
    out = []

    smallest = sizes_desc[-1]

    def rec(i, left, k, acc):
        if k == 0:
            if left == 0:
                out.append(tuple(acc))
            return
        for j in range(i, len(sizes_desc)):
            v = sizes_desc[j]
            if v * k < left:
                break  # v (and everything after) too small to reach target
            if left - v < smallest * (k - 1):
                continue  # v too big
            acc.append(v)
            rec(j, left - v, k - 1, acc)
            acc.pop()
    rec(0, target, S, [])
    return out


@lru_cache(maxsize=None)
def _find_structure(counts):
    """Pick slot sizes minimizing per-core capacity: scan capacity targets
    ascending, and for each try all slot-size multisets hitting it."""
    counts = tuple(int(c) for c in counts)
    total = sum(counts)
    # every expert over-covers by at least (-c) % GRAN given granular sizes
    min_sum = -(-(total + sum((-c) % _GRAN for c in counts)) // 8)
    min_sum = -(-min_sum // _GRAN) * _GRAN
    sizes_desc = sorted(_SIZES, reverse=True)
    for target in range(max(min_sum, 1024), 2433, _GRAN):
        cands = []
        for S in (2, 3, 4, 5):
            cands.extend(_multisets_summing(sizes_desc, S, target))
        cands.sort(key=lambda Ls: (len(Ls), -min(Ls)))
        for Ls in cands[:3000]:
            Ls = tuple(sorted(Ls))
            asg = _try_assign(Ls, counts)
            if asg is not None:
                return Ls, asg
    return _FALLBACK, _try_assign(_FALLBACK, counts)


def _make_plan(counts):
    """Return (Ls, chunks) where chunks[core][slot] = (expert, t0, n)."""
    Ls, asg = _find_structure(tuple(int(c) for c in counts))
    S = len(Ls)
    offs = np.concatenate([[0], np.cumsum(counts)]).astype(np.int64)
    per_class = [[] for _ in range(S)]
    for e, ns in enumerate(asg):
        pos = int(offs[e])
        remaining = int(counts[e])
        # fill largest class chunks first
        for k in sorted(range(S), key=lambda k: -Ls[k]):
            for _ in range(ns[k]):
                take = min(remaining, Ls[k])
                per_class[k].append((e, pos, take))
                pos += take
                remaining -= take
        assert remaining == 0
    chunks = [[None] * S for _ in range(NCORES)]
    for k in range(S):
        cl = per_class[k]
        assert len(cl) <= NCORES
        for j in range(NCORES):
            chunks[j][k] = cl[j] if j < len(cl) else (-1, 0, 0)
    return Ls, chunks


def _nchunks(L):
    # PSUM-bank-aligned matmul column chunks: a bank holds 512 fp32, so any
    # chunk must not cross a 512-column boundary.
    out, n0 = [], 0
    while L - n0 > 512:
        out.append((n0, 512))
        n0 += 512
    out.append((n0, L - n0))
    return out


@lru_cache(maxsize=4)
def _build_program(Ls):
    import concourse.bacc as bacc
    import concourse.tile as tile
    from concourse import mybir

    f32 = mybir.dt.float32
    bf16 = mybir.dt.bfloat16
    nc = bacc.Bacc("TRN2", target_bir_lowering=False, debug=False,
                   num_devices=NCORES, name="moe_experts")

    xt_d, w1_d, w3_d, w2_d, out_d = [], [], [], [], []
    for s, L in enumerate(Ls):
        xt_d.append(nc.dram_tensor(f"xt{s}", (P, DC, L), bf16, kind="ExternalInput"))
        w1_d.append(nc.dram_tensor(f"w1r{s}", (HC, P, DC, P), bf16, kind="ExternalInput"))
        w3_d.append(nc.dram_tensor(f"w3r{s}", (HC, P, DC, P), bf16, kind="ExternalInput"))
        w2_d.append(nc.dram_tensor(f"w2r{s}", (DC, P, HC, P), bf16, kind="ExternalInput"))
        out_d.append(nc.dram_tensor(f"out{s}", (DC, P, L), f32, kind="ExternalOutput"))

    with tile.TileContext(nc) as tc:
        with (
            tc.tile_pool(name="xpool", bufs=2) as xpool,
            tc.tile_pool(name="hpool", bufs=2) as hpool,
            tc.tile_pool(name="wpool", bufs=3) as wpool,
            tc.tile_pool(name="spool", bufs=2) as spool,
            tc.tile_pool(name="psum", bufs=2, space="PSUM") as psum,
        ):
            for s, L in enumerate(Ls):
                nch = _nchunks(L)
                # bulk x loads ride the Act HWDGE ring so their descriptor
                # bursts never queue ahead of the per-hc weight stream (SP ring)
                xt = xpool.tile([P, DC, L], bf16, tag="xt")
                if s == 0:
                    # fine-grained so the very first matmuls start ASAP
                    for dc in range(DC):
                        nc.scalar.dma_start(xt[:, dc, :], xt_d[s].ap()[:, dc, :])
                else:
                    half = DC // 2
                    nc.scalar.dma_start(xt[:, :half, :], xt_d[s].ap()[:, :half, :])
                    nc.scalar.dma_start(xt[:, half:, :], xt_d[s].ap()[:, half:, :])
                hbuf = hpool.tile([P, HC, L], bf16, tag="h")
                for hc in range(HC):
                    w1t = wpool.tile([P, DC, P], bf16, tag="w1")
                    nc.sync.dma_start(w1t[:], w1_d[s].ap()[hc])
                    w3t = wpool.tile([P, DC, P], bf16, tag="w3")
                    nc.sync.dma_start(w3t[:], w3_d[s].ap()[hc])
                    p1 = psum.tile([P, L], f32, tag="p1")
                    p3 = psum.tile([P, L], f32, tag="p3")
                    # dc outer / column-chunk inner: consecutive matmuls share
                    # the stationary weight tile
                    for dc in range(DC):
                        for (n0, nsz) in nch:
                            nc.tensor.matmul(
                                p1[:, n0:n0 + nsz],
                                w1t[:, dc, :],
                                xt[:, dc, n0:n0 + nsz],
                                start=(dc == 0), stop=(dc == DC - 1),
                            )
                    for dc in range(DC):
                        for (n0, nsz) in nch:
                            nc.tensor.matmul(
                                p3[:, n0:n0 + nsz],
                                w3t[:, dc, :],
                                xt[:, dc, n0:n0 + nsz],
                                start=(dc == 0), stop=(dc == DC - 1),
                            )
                    stmp = spool.tile([P, L], f32, tag="stmp")
                    nc.scalar.activation(stmp[:], p1[:], mybir.ActivationFunctionType.Silu)
                    nc.vector.tensor_mul(out=hbuf[:, hc, :], in0=stmp[:], in1=p3[:])
                for dc in range(DC):
                    w2t = wpool.tile([P, HC, P], bf16, tag="w2")
                    nc.sync.dma_start(w2t[:], w2_d[s].ap()[dc])
                    po = psum.tile([P, L], f32, tag="p1")
                    for hc in range(HC):
                        for (n0, nsz) in nch:
                            nc.tensor.matmul(
                                po[:, n0:n0 + nsz],
                                w2t[:, hc, :],
                                hbuf[:, hc, n0:n0 + nsz],
                                start=(hc == 0), stop=(hc == HC - 1),
                            )
                    ot = spool.tile([P, L], f32, tag="ot")
                    nc.any.tensor_copy(out=ot[:], in_=po[:])
                    nc.scalar.dma_start(out_d[s].ap()[dc], ot[:])

    nc.compile()
    return nc


def _permute_w13(w):  # [H, D] -> [HC, P(k=d), DC, P(m=h)]
    return np.ascontiguousarray(
        w.reshape(HC, P, DC, P).transpose(0, 3, 2, 1).astype(BF16))


def _permute_w2(w):  # [D, H] -> [DC, P(k=h), HC, P(m=d)]
    return np.ascontiguousarray(
        w.reshape(DC, P, HC, P).transpose(0, 3, 2, 1).astype(BF16))


def kernel(x, w1, w2, w3, num_tokens_per_expert):
    from concourse.bass_utils import run_bass_kernel_spmd

    x = np.asarray(x, dtype=np.float32)
    w1 = np.asarray(w1, dtype=np.float32)
    w2 = np.asarray(w2, dtype=np.float32)
    w3 = np.asarray(w3, dtype=np.float32)
    counts = np.asarray(num_tokens_per_expert).astype(np.int64)

    Ls, chunks = _make_plan(counts)
    nc = _build_program(tuple(Ls))

    experts_used = sorted({e for row in chunks for (e, _, _) in row if e >= 0})
    if not experts_used:
        experts_used = [0]
    w1r = {e: _permute_w13(w1[e]) for e in experts_used}
    w3r = {e: _permute_w13(w3[e]) for e in experts_used}
    w2r = {e: _permute_w2(w2[e]) for e in experts_used}
    e_dummy = experts_used[0]

    xb = x.astype(BF16)
    in_maps = []
    for c in range(NCORES):
        m = {}
        for s, L in enumerate(Ls):
            e, t0, n = chunks[c][s]
            if e < 0:
                e = e_dummy
            xs = np.zeros((L, D), dtype=BF16)
            if n:
                xs[:n] = xb[t0:t0 + n]
            # [L, D] -> [P(k=d), DC, L]
            m[f"xt{s}"] = np.ascontiguousarray(
                xs.reshape(L, DC, P).transpose(2, 1, 0))
            m[f"w1r{s}"] = w1r[e]
            m[f"w3r{s}"] = w3r[e]
            m[f"w2r{s}"] = w2r[e]
        in_maps.append(m)

    res = run_bass_kernel_spmd(nc, in_maps, core_ids=list(range(NCORES)))

    out = np.zeros((T, D), dtype=np.float32)
    for c in range(NCORES):
        for s in range(len(Ls)):
            e, t0, n = chunks[c][s]
            if e < 0 or n == 0:
                continue
            o = res.results[c][f"out{s}"]  # [DC, P, L]
            out[t0:t0 + n] = o[:, :, :n].transpose(2, 0, 1).reshape(n, D)
    return out


# revision 12
# speedup vs baseline: 1.0991x; 1.0991x over previous
"""Grouped-experts MoE FFN (SwiGLU) kernel for Trainium2, 8 NeuronCores.

Strategy: expert-parallel with host-side packing.  Token counts per expert
are data, so the host chops each expert's contiguous token block into
chunks and bins them into a uniform per-core "slot" structure
(S slots per core, compile-time sizes).  Every slot is bound to exactly
one expert; the expert's (host-pre-permuted) weights are plain kernel
inputs, so the SPMD program is identical on all 8 cores and needs no
device-side transposes or gather/scatter.

All matmul operands are bf16 (PSUM accumulates fp32): same PE throughput
as fp32r (1 col/cycle) but half the DMA traffic and SBUF footprint, and
the compiler's fast-weight-load path (disabled for fp32) hides LDWEIGHTS
under the matmul stream.

Per-core device program, per slot of capacity L (tokens):
  phase A: for each 128-row h-chunk (22 of them):
      psum1[128,L] = sum_dc w1r[hc,dc].T @ xT[dc]     (bf16 matmuls)
      psum3[128,L] = sum_dc w3r[hc,dc].T @ xT[dc]
      h[hc] = silu(psum1) * psum3                     (ACT + DVE, bf16 out)
  phase B: for each 128-row d-chunk (8):
      po[128,L] = sum_hc w2r[dc,hc].T @ h[hc]
      DMA po -> outT[dc]                              ([D,L] layout, fp32)

Host then transposes each slot's [D, L] output back and scatters into the
full [T, D] result (padding rows stay zero).
"""

import itertools
import numpy as np
from functools import lru_cache

import ml_dtypes

E, D, H, T = 8, 1024, 2816, 16384
P = 128
DC, HC = D // P, H // P  # 8, 22
NCORES = 8
BF16 = ml_dtypes.bfloat16

# Slot sizes: multiples of 16.  Measured on HW: matmul chunks below ~300
# columns pay per-instruction overhead and starve the per-hc weight-DMA
# stream, and chunks in (512, 704) would leave a trailing chunk shorter
# than an LDWEIGHTS.  So slots are either single-chunk [320, 512] or
# two-chunk [704, 1024] (512 + >=192).
_SIZES = [s for s in range(320, 1025, 16) if not (512 < s < 704)]
_GRAN = 16
_FALLBACK = (1024, 1024, 1024)  # feasible for any counts with sum <= T


def _try_assign(Ls, counts, node_cap=500000):
    """Greedy: assign experts (largest first) chunk counts per size class
    (<= NCORES slots per class), minimizing each expert's over-capacity via
    bounded branch&bound.  Returns per-expert chunk-count tuples or None."""
    S = len(Ls)
    avail = [NCORES] * S
    out = [None] * len(counts)
    order = sorted(range(len(counts)), key=lambda e: -counts[e])
    nodes = [0]
    for e in order:
        g = counts[e]
        best = None

        def rec(k, ns, cap):
            nonlocal best
            nodes[0] += 1
            if nodes[0] > node_cap:
                return
            if cap >= g:
                if best is None or cap - g < best[0]:
                    best = (cap - g, tuple(ns))
                return
            if k == S:
                return
            if best is not None and best[0] == 0:
                return
            if cap + sum(avail[j] * Ls[j] for j in range(k, S)) < g:
                return
            for n in range(avail[k], -1, -1):
                rec(k + 1, ns + [n], cap + n * Ls[k])

        rec(0, [], 0)
        if best is None:
            return None
        ns = best[1] + (0,) * (S - len(best[1]))
        for k in range(S):
            avail[k] -= ns[k]
        out[e] = ns
    return out


def _multisets_summing(sizes_desc, S, target):
    """All descending multisets of length S from sizes_desc with given sum."""
    out = []

    smallest = sizes_desc[-1]

    def rec(i, left, k, acc):
        if k == 0:
            if left == 0:
                out.append(tuple(acc))
            return
        for j in range(i, len(sizes_desc)):
            v = sizes_desc[j]
            if v * k < left:
                break  # v (and everything after) too small to reach target
            if left - v < smallest * (k - 1):
                continue  # v too big
            acc.append(v)
            rec(j, left - v, k - 1, acc)
            acc.pop()
    rec(0, target, S, [])
    return out


@lru_cache(maxsize=None)
def _find_structure(counts):
    """Pick slot sizes minimizing per-core capacity: scan capacity targets
    ascending, and for each try all slot-size multisets hitting it."""
    counts = tuple(int(c) for c in counts)
    total = sum(counts)
    # every expert over-covers by at least (-c) % GRAN given granular sizes
    min_sum = -(-(total + sum((-c) % _GRAN for c in counts)) // 8)
    min_sum = -(-min_sum // _GRAN) * _GRAN
    sizes_desc = sorted(_SIZES, reverse=True)
    for target in range(max(min_sum, 1024), 2433, _GRAN):
        cands = []
        for S in (2, 3, 4, 5, 6):
            cands.extend(_multisets_summing(sizes_desc, S, target))
        # fewer matmul chunks -> less per-instruction overhead on the PE
        cands.sort(key=lambda Ls: (sum(len(_nchunks(L)) for L in Ls),
                                   len(Ls), -min(Ls)))
        for Ls in cands[:3000]:
            Ls = tuple(sorted(Ls, reverse=True))
            asg = _try_assign(Ls, counts)
            if asg is not None:
                return Ls, asg
    return _FALLBACK, _try_assign(_FALLBACK, counts)


def _make_plan(counts):
    """Return (Ls, chunks) where chunks[core][slot] = (expert, t0, n)."""
    Ls, asg = _find_structure(tuple(int(c) for c in counts))
    S = len(Ls)
    offs = np.concatenate([[0], np.cumsum(counts)]).astype(np.int64)
    per_class = [[] for _ in range(S)]
    for e, ns in enumerate(asg):
        pos = int(offs[e])
        remaining = int(counts[e])
        # fill largest class chunks first
        for k in sorted(range(S), key=lambda k: -Ls[k]):
            for _ in range(ns[k]):
                take = min(remaining, Ls[k])
                per_class[k].append((e, pos, take))
                pos += take
                remaining -= take
        assert remaining == 0
    chunks = [[None] * S for _ in range(NCORES)]
    for k in range(S):
        cl = per_class[k]
        assert len(cl) <= NCORES
        for j in range(NCORES):
            chunks[j][k] = cl[j] if j < len(cl) else (-1, 0, 0)
    return Ls, chunks


def _nchunks(L):
    # PSUM-bank-aligned matmul column chunks: a bank holds 512 fp32, so any
    # chunk must not cross a 512-column boundary.
    out, n0 = [], 0
    while L - n0 > 512:
        out.append((n0, 512))
        n0 += 512
    out.append((n0, L - n0))
    return out


@lru_cache(maxsize=4)
def _build_program(Ls):
    import concourse.bacc as bacc
    import concourse.tile as tile
    from concourse import mybir

    f32 = mybir.dt.float32
    bf16 = mybir.dt.bfloat16
    nc = bacc.Bacc("TRN2", target_bir_lowering=False, debug=False,
                   num_devices=NCORES, name="moe_experts")

    xt_d, w1_d, w3_d, w2_d, out_d = [], [], [], [], []
    for s, L in enumerate(Ls):
        xt_d.append(nc.dram_tensor(f"xt{s}", (P, DC, L), bf16, kind="ExternalInput"))
        w1_d.append(nc.dram_tensor(f"w1r{s}", (HC, P, DC, P), bf16, kind="ExternalInput"))
        w3_d.append(nc.dram_tensor(f"w3r{s}", (HC, P, DC, P), bf16, kind="ExternalInput"))
        w2_d.append(nc.dram_tensor(f"w2r{s}", (DC, P, HC, P), bf16, kind="ExternalInput"))
        out_d.append(nc.dram_tensor(f"out{s}", (DC, P, L), f32, kind="ExternalOutput"))

    with tile.TileContext(nc) as tc:
        with (
            tc.tile_pool(name="xpool", bufs=2) as xpool,
            tc.tile_pool(name="hpool", bufs=2) as hpool,
            tc.tile_pool(name="wpool", bufs=3) as wpool,
            tc.tile_pool(name="spool", bufs=2) as spool,
            tc.tile_pool(name="psum", bufs=2, space="PSUM") as psum,
        ):
            for s, L in enumerate(Ls):
                nch = _nchunks(L)
                # bulk x loads ride the Act HWDGE ring so their descriptor
                # bursts never queue ahead of the per-hc weight stream (SP ring)
                xt = xpool.tile([P, DC, L], bf16, tag="xt")
                if s == 0:
                    # fine-grained so the very first matmuls start ASAP
                    for dc in range(DC):
                        nc.scalar.dma_start(xt[:, dc, :], xt_d[s].ap()[:, dc, :])
                else:
                    half = DC // 2
                    nc.scalar.dma_start(xt[:, :half, :], xt_d[s].ap()[:, :half, :])
                    nc.scalar.dma_start(xt[:, half:, :], xt_d[s].ap()[:, half:, :])
                hbuf = hpool.tile([P, HC, L], bf16, tag="h")
                for hc in range(HC):
                    w1t = wpool.tile([P, DC, P], bf16, tag="w1")
                    nc.sync.dma_start(w1t[:], w1_d[s].ap()[hc])
                    w3t = wpool.tile([P, DC, P], bf16, tag="w3")
                    nc.sync.dma_start(w3t[:], w3_d[s].ap()[hc])
                    p1 = psum.tile([P, L], f32, tag="p1")
                    p3 = psum.tile([P, L], f32, tag="p3")
                    # dc outer / column-chunk inner: consecutive matmuls share
                    # the stationary weight tile
                    for dc in range(DC):
                        for (n0, nsz) in nch:
                            nc.tensor.matmul(
                                p1[:, n0:n0 + nsz],
                                w1t[:, dc, :],
                                xt[:, dc, n0:n0 + nsz],
                                start=(dc == 0), stop=(dc == DC - 1),
                            )
                    for dc in range(DC):
                        for (n0, nsz) in nch:
                            nc.tensor.matmul(
                                p3[:, n0:n0 + nsz],
                                w3t[:, dc, :],
                                xt[:, dc, n0:n0 + nsz],
                                start=(dc == 0), stop=(dc == DC - 1),
                            )
                    stmp = spool.tile([P, L], f32, tag="stmp")
                    nc.scalar.activation(stmp[:], p1[:], mybir.ActivationFunctionType.Silu)
                    nc.vector.tensor_mul(out=hbuf[:, hc, :], in0=stmp[:], in1=p3[:])
                for dc in range(DC):
                    w2t = wpool.tile([P, HC, P], bf16, tag="w2")
                    nc.sync.dma_start(w2t[:], w2_d[s].ap()[dc])
                    po = psum.tile([P, L], f32, tag="p1")
                    for hc in range(HC):
                        for (n0, nsz) in nch:
                            nc.tensor.matmul(
                                po[:, n0:n0 + nsz],
                                w2t[:, hc, :],
                                hbuf[:, hc, n0:n0 + nsz],
                                start=(hc == 0), stop=(hc == HC - 1),
                            )
                    ot = spool.tile([P, L], f32, tag="ot")
                    nc.any.tensor_copy(out=ot[:], in_=po[:])
                    nc.scalar.dma_start(out_d[s].ap()[dc], ot[:])

    nc.compile()
    return nc


def _permute_w13(w):  # [H, D] -> [HC, P(k=d), DC, P(m=h)]
    return np.ascontiguousarray(
        w.reshape(HC, P, DC, P).transpose(0, 3, 2, 1).astype(BF16))


def _permute_w2(w):  # [D, H] -> [DC, P(k=h), HC, P(m=d)]
    return np.ascontiguousarray(
        w.reshape(DC, P, HC, P).transpose(0, 3, 2, 1).astype(BF16))


def kernel(x, w1, w2, w3, num_tokens_per_expert):
    from concourse.bass_utils import run_bass_kernel_spmd

    x = np.asarray(x, dtype=np.float32)
    w1 = np.asarray(w1, dtype=np.float32)
    w2 = np.asarray(w2, dtype=np.float32)
    w3 = np.asarray(w3, dtype=np.float32)
    counts = np.asarray(num_tokens_per_expert).astype(np.int64)

    Ls, chunks = _make_plan(counts)
    nc = _build_program(tuple(Ls))

    experts_used = sorted({e for row in chunks for (e, _, _) in row if e >= 0})
    if not experts_used:
        experts_used = [0]
    w1r = {e: _permute_w13(w1[e]) for e in experts_used}
    w3r = {e: _permute_w13(w3[e]) for e in experts_used}
    w2r = {e: _permute_w2(w2[e]) for e in experts_used}
    e_dummy = experts_used[0]

    xb = x.astype(BF16)
    in_maps = []
    for c in range(NCORES):
        m = {}
        for s, L in enumerate(Ls):
            e, t0, n = chunks[c][s]
            if e < 0:
                e = e_dummy
            xs = np.zeros((L, D), dtype=BF16)
            if n:
                xs[:n] = xb[t0:t0 + n]
            # [L, D] -> [P(k=d), DC, L]
            m[f"xt{s}"] = np.ascontiguousarray(
                xs.reshape(L, DC, P).transpose(2, 1, 0))
            m[f"w1r{s}"] = w1r[e]
            m[f"w3r{s}"] = w3r[e]
            m[f"w2r{s}"] = w2r[e]
        in_maps.append(m)

    res = run_bass_kernel_spmd(nc, in_maps, core_ids=list(range(NCORES)))

    out = np.zeros((T, D), dtype=np.float32)
    for c in range(NCORES):
        for s in range(len(Ls)):
            e, t0, n = chunks[c][s]
            if e < 0 or n == 0:
                continue
            o = res.results[c][f"out{s}"]  # [DC, P, L]
            out[t0:t0 + n] = o[:, :, :n].transpose(2, 0, 1).reshape(n, D)
    return out
